# revision 2
# baseline (speedup 1.0000x reference)
"""Trainium2 Bass kernel for nn_DistanceLayer (gaussian-prior distance attention).

Math: out[b,i] = sum_j softmax_j(q_i.k_j * MD^-0.5 * prior(j-i))[j] * (j-i)

Key observation: the gaussian prior (std=1) underflows so fast in f32 that
for |j-i| outside a small band the f32 score is exactly 0, so exp(score)
is exactly 1.0.  The softmax row then consists of a small band of
"interesting" values plus a uniform far field whose sums are known in
closed form.  We therefore compute, per 128-row tile, only a 160-column
window of scores around the diagonal on the PE, and fold the far field in
with exact host-side constants:

    T0_i = (N - win) + sum_window exp(s)            (denominator)
    T1_i = C1_t + sum_window exp(s)*c + ws_t * sum_window exp(s)
    out_i = T1_i / T0_i - i

where C1_t = sum_all_j j - sum_window j (exact integers < 2^24, exact in
f32) and ws_t is the window start of tile t.  In-window far entries have
score exactly 0 (prior premultiplied in, 0 outside band) and contribute
exp(0)=1 which the constants account for.

Sharding: pure data-parallel over batch B=8 across the 8 cores; each core
holds the full (small) QK weights and computes its own [N] output row.
"""

import sys

sys.path.insert(0, "/opt/trn_rl_repo")

import ml_dtypes
import numpy as np

import concourse.bacc as bacc
import concourse.tile as tile
from concourse import mybir
from concourse.bass_utils import run_bass_kernel_spmd

B, N, D, MD = 8, 2048, 256, 128
NCORES = 8
P = 128
NT = N // P  # 16 row tiles
DCH = D // P  # 2 contraction chunks for the projections
PROJ_CHUNK = 512
NPC = N // PROJ_CHUNK  # 4 projection column chunks
PI = 3.1415926  # matches reference
F32 = mybir.dt.float32
BF16 = mybir.dt.bfloat16

_cache = {}
# exposed for test harness profiling: (nc, in_maps)
last_run = None


def _plan_band(prior_mean, prior_std):
    """f32 prior over every offset, exactly as the reference computes it,
    and the band of offsets whose scores can round exp() away from 1.0."""
    d = np.arange(-(N - 1), N, dtype=np.float32)
    ps = np.float32(prior_std)
    pm = np.float32(prior_mean)
    prior = (
        np.float32(1.0)
        / ps
        / np.sqrt(np.float32(2.0) * np.float32(PI))
        * np.exp(np.float32(-0.5) * (d - pm) ** 2 / ps**2)
    ).astype(np.float32)
    # |score| <= |prior| * |q.k*scale| ; bound the latter by 1024 (actual
    # max is ~7 for these glorot inputs).  exp(x) rounds to 1.0f for
    # |x| < 2^-26; use 2^-27 for margin.
    sig = np.abs(prior) * 1024.0 >= 2.0**-27
    if not sig.any():
        dlo, dhi = 0, 0
    else:
        dlo = int(d[sig].min())
        dhi = int(d[sig].max())
    return prior, dlo, dhi


def _window_geometry(dlo, dhi):
    span = dhi - dlo
    win = 128 + span + 16
    win = max(160, ((win + 31) // 32) * 32)
    assert win <= 512, f"prior band too wide for banded kernel: {dlo}..{dhi}"
    extra = win - (128 + span)
    ws_list = []
    for t in range(NT):
        ws = min(max(t * P + dlo - extra // 2, 0), N - win)
        lo_need = max(0, t * P + dlo)
        hi_need = min(N - 1, t * P + 127 + dhi)
        assert ws <= lo_need and hi_need < ws + win, (t, ws, lo_need, hi_need)
        ws_list.append(ws)
    off_vals = sorted({t * P - ws_list[t] for t in range(NT)})
    off_idx = [off_vals.index(t * P - ws_list[t]) for t in range(NT)]
    return win, ws_list, off_vals, off_idx


def _build(win, ws_list, off_idx, n_off):
    nc = bacc.Bacc()

    xt_d = nc.dram_tensor("xt", [DCH * NPC, P, PROJ_CHUNK], BF16, kind="ExternalInput")
    wq_d = nc.dram_tensor("wq", [P, DCH * MD], BF16, kind="ExternalInput")
    wk_d = nc.dram_tensor("wk", [P, DCH * MD], BF16, kind="ExternalInput")
    bq_d = nc.dram_tensor("bq", [P, 1], F32, kind="ExternalInput")
    bk_d = nc.dram_tensor("bk", [P, 1], F32, kind="ExternalInput")
    pm_d = nc.dram_tensor("pmat", [P, n_off * win], F32, kind="ExternalInput")
    j0_d = nc.dram_tensor("j0", [P, win], F32, kind="ExternalInput")
    c1_d = nc.dram_tensor("c1", [P, NT], F32, kind="ExternalInput")
    ws_d = nc.dram_tensor("wsm", [P, NT], F32, kind="ExternalInput")
    ii_d = nc.dram_tensor("ii", [P, NT], F32, kind="ExternalInput")
    y_d = nc.dram_tensor("y", [N], F32, kind="ExternalOutput")

    with tile.TileContext(nc) as tc:
        with (
            tc.tile_pool(name="const", bufs=1) as const,
            tc.tile_pool(name="psum_proj", bufs=3, space="PSUM") as psum_proj,
            tc.tile_pool(name="psum_band", bufs=4, space="PSUM") as psum_band,
            tc.tile_pool(name="band_sp", bufs=3) as sp_pool,
            tc.tile_pool(name="band_e", bufs=3) as e_pool,
            tc.tile_pool(name="band_ec", bufs=3) as ec_pool,
            tc.tile_pool(name="comb", bufs=1) as comb,
        ):
            # ---- input DMAs ----
            xts = []
            for i in range(DCH * NPC):
                t = const.tile([P, PROJ_CHUNK], BF16, tag=f"xt{i}")
                nc.sync.dma_start(out=t, in_=xt_d[i])
                xts.append(t)
            wq_s = const.tile([P, DCH * MD], BF16, tag="wq")
            nc.sync.dma_start(out=wq_s, in_=wq_d[:, :])
            wk_s = const.tile([P, DCH * MD], BF16, tag="wk")
            nc.sync.dma_start(out=wk_s, in_=wk_d[:, :])
            bq_s = const.tile([P, 1], F32, tag="bq")
            nc.sync.dma_start(out=bq_s, in_=bq_d[:, :])
            bk_s = const.tile([P, 1], F32, tag="bk")
            nc.sync.dma_start(out=bk_s, in_=bk_d[:, :])
            pm_s = const.tile([P, n_off * win], F32, tag="pmat")
            nc.sync.dma_start(out=pm_s, in_=pm_d[:, :])
            j0_s = const.tile([P, win], F32, tag="j0")
            nc.sync.dma_start(out=j0_s, in_=j0_d[:, :])
            c1_s = const.tile([P, NT], F32, tag="c1")
            nc.sync.dma_start(out=c1_s, in_=c1_d[:, :])
            ws_s = const.tile([P, NT], F32, tag="wsm")
            nc.sync.dma_start(out=ws_s, in_=ws_d[:, :])
            ii_s = const.tile([P, NT], F32, tag="ii")
            nc.sync.dma_start(out=ii_s, in_=ii_d[:, :])

            qT = const.tile([P, N], BF16, tag="qT")
            kT = const.tile([P, N], BF16, tag="kT")
            sum_e = const.tile([P, NT], F32, tag="sum_e")
            sum_ec = const.tile([P, NT], F32, tag="sum_ec")

            # ---- projections: qT = Wq.T @ x.T (+bq), kT likewise ----
            for w_s, b_s, dstT in ((wq_s, bq_s, qT), (wk_s, bk_s, kT)):
                for n4 in range(NPC):
                    ps_t = psum_proj.tile([P, PROJ_CHUNK], F32, tag="proj")
                    for c in range(DCH):
                        nc.tensor.matmul(
                            ps_t,
                            lhsT=w_s[:, c * MD : (c + 1) * MD],
                            rhs=xts[c * NPC + n4],
                            start=(c == 0),
                            stop=(c == DCH - 1),
                        )
                    nc.scalar.activation(
                        out=dstT[:, n4 * PROJ_CHUNK : (n4 + 1) * PROJ_CHUNK],
                        in_=ps_t,
                        func=mybir.ActivationFunctionType.Identity,
                        bias=b_s,
                        scale=1.0,
                    )

            # ---- banded scores + softmax partial sums ----
            for t in range(NT):
                ws = ws_list[t]
                oi = off_idx[t]
                ps_s = psum_band.tile([P, win], F32, tag="band")
                nc.tensor.matmul(
                    ps_s,
                    lhsT=qT[:, t * P : (t + 1) * P],
                    rhs=kT[:, ws : ws + win],
                    start=True,
                    stop=True,
                )
                sp_t = sp_pool.tile([P, win], F32, tag="sp")
                nc.vector.tensor_mul(sp_t, ps_s, pm_s[:, oi * win : (oi + 1) * win])
                e_t = e_pool.tile([P, win], F32, tag="e")
                nc.scalar.activation(
                    out=e_t,
                    in_=sp_t,
                    func=mybir.ActivationFunctionType.Exp,
                    accum_out=sum_e[:, t : t + 1],
                )
                ec_t = ec_pool.tile([P, win], F32, tag="ec")
                nc.vector.affine_mul_reduce(
                    out=ec_t,
                    accum_out=sum_ec[:, t : t + 1],
                    in0=e_t,
                    in1=j0_s,
                    scale=1.0,
                    bias=0.0,
                )

            # ---- combine: out = (c1 + sum_ec + ws*sum_e)/(N-win+sum_e) - i ----
            t0 = comb.tile([P, NT], F32, tag="t0")
            nc.vector.tensor_scalar_add(t0, sum_e, float(N - win))
            rec = comb.tile([P, NT], F32, tag="rec")
            nc.vector.reciprocal(rec, t0)
            tmp = comb.tile([P, NT], F32, tag="tmp")
            nc.vector.tensor_mul(tmp, ws_s, sum_e)
            num = comb.tile([P, NT], F32, tag="num")
            nc.vector.tensor_add(num, c1_s, sum_ec)
            num2 = comb.tile([P, NT], F32, tag="num2")
            nc.vector.tensor_add(num2, num, tmp)
            outv = comb.tile([P, NT], F32, tag="outv")
            nc.vector.tensor_mul(outv, num2, rec)
            outv2 = comb.tile([P, NT], F32, tag="outv2")
            nc.vector.tensor_sub(outv2, outv, ii_s)

            y_ap = y_d[:].rearrange("(t p) -> p t", p=P)
            nc.sync.dma_start(out=y_ap, in_=outv2)

    nc.finalize()
    return nc


def kernel(x, Wq, bq, Wk, bk, prior_mean, prior_std):
    global last_run
    x = np.asarray(x, dtype=np.float32)
    Wq = np.asarray(Wq, dtype=np.float32)
    Wk = np.asarray(Wk, dtype=np.float32)
    bq = np.asarray(bq, dtype=np.float32)
    bk = np.asarray(bk, dtype=np.float32)

    prior, dlo, dhi = _plan_band(float(np.asarray(prior_mean)[0]), float(np.asarray(prior_std)[0]))
    win, ws_list, off_vals, off_idx = _window_geometry(dlo, dhi)
    n_off = len(off_vals)

    key = (win, tuple(ws_list), tuple(off_idx))
    if key not in _cache:
        _cache[key] = _build(win, ws_list, off_idx, n_off)
    nc = _cache[key]

    bf = ml_dtypes.bfloat16
    scale = np.float32(MD**-0.5)

    # prior*scale window matrices, one per distinct tile offset
    r = np.arange(P)[:, None]
    c = np.arange(win)[None, :]
    pmat = np.zeros((P, n_off * win), np.float32)
    for k_i, off in enumerate(off_vals):
        dm = c - off - r
        pmat[:, k_i * win : (k_i + 1) * win] = np.where(
            (dm >= dlo) & (dm <= dhi), prior[dm + N - 1] * scale, np.float32(0.0)
        ).astype(np.float32)

    j0 = np.broadcast_to(np.arange(win, dtype=np.float32), (P, win)).copy()
    sumj_all = float(N * (N - 1) // 2)
    c1 = np.zeros((P, NT), np.float32)
    wsm = np.zeros((P, NT), np.float32)
    ii = np.zeros((P, NT), np.float32)
    for t in range(NT):
        ws = ws_list[t]
        c1[:, t] = sumj_all - (win * ws + win * (win - 1) // 2)
        wsm[:, t] = ws
        ii[:, t] = t * P + np.arange(P)

    wq_h = np.ascontiguousarray(
        Wq.reshape(DCH, P, MD).transpose(1, 0, 2).reshape(P, DCH * MD)
    ).astype(bf)
    wk_h = np.ascontiguousarray(
        Wk.reshape(DCH, P, MD).transpose(1, 0, 2).reshape(P, DCH * MD)
    ).astype(bf)
    bq_h = np.ascontiguousarray(bq.reshape(P, 1))
    bk_h = np.ascontiguousarray(bk.reshape(P, 1))

    in_maps = []
    for core in range(NCORES):
        xb = x[core]  # [N, D]
        # xt[c*NPC + n4, p, j] = x[n4*512 + j, c*128 + p]
        xt_h = np.ascontiguousarray(
            xb.T.reshape(DCH, P, NPC, PROJ_CHUNK)
            .transpose(0, 2, 1, 3)
            .reshape(DCH * NPC, P, PROJ_CHUNK)
        ).astype(bf)
        in_maps.append(
            {
                "xt": xt_h,
                "wq": wq_h,
                "wk": wk_h,
                "bq": bq_h,
                "bk": bk_h,
                "pmat": pmat,
                "j0": j0,
                "c1": c1,
                "wsm": wsm,
                "ii": ii,
            }
        )

    res = run_bass_kernel_spmd(nc, in_maps, list(range(NCORES)))
    last_run = (nc, in_maps)
    out = np.stack([res.results[c]["y"] for c in range(NCORES)], axis=0)
    return out.astype(np.float32)


# revision 4
# speedup vs baseline: 1.3017x; 1.3017x over previous
"""Trainium2 Bass kernel for nn_DistanceLayer (gaussian-prior distance attention).

Math: out[b,i] = sum_j softmax_j(q_i.k_j * MD^-0.5 * prior(j-i))[j] * (j-i)

Key observation: the gaussian prior (std=1) underflows so fast in f32 that
for |j-i| outside a small band the f32 score is exactly 0, so exp(score)
is exactly 1.0.  The softmax row then consists of a small band of
"interesting" values plus a uniform far field whose sums are known in
closed form.  We therefore compute, per 128-row tile, only a small window
of scores around the diagonal on the PE, and fold the far field in with
exact host-side constants:

    T0_i = (N - win) + sum_window exp(s)            (denominator)
    T1_i = C1_t + sum_window exp(s)*c + ws_t * sum_window exp(s)
    out_i = T1_i / T0_i - i

where C1_t = sum_all_j j - sum_window j (exact integers < 2^24, exact in
f32) and ws_t is the window start of tile t.  In-window far entries have
score exactly 0 (prior premultiplied in, 0 outside band) and contribute
exp(0)=1 which the constants account for.

Sharding: pure data-parallel over batch B=8 across the 8 cores; each core
holds the full (small) QK weights and computes its own [N] output row.
"""

import sys

sys.path.insert(0, "/opt/trn_rl_repo")

import ml_dtypes
import numpy as np

import concourse.bacc as bacc
import concourse.tile as tile
from concourse import mybir
from concourse.bass_utils import run_bass_kernel_spmd

B, N, D, MD = 8, 2048, 256, 128
NCORES = 8
P = 128
NT = N // P  # 16 row tiles
DCH = D // P  # 2 contraction chunks for the projections
PROJ_CHUNK = 512
NPC = N // PROJ_CHUNK  # 4 projection column chunks
PI = 3.1415926  # matches reference
F32 = mybir.dt.float32
BF16 = mybir.dt.bfloat16

_cache = {}
# exposed for test harness profiling: (nc, in_maps)
last_run = None


def _plan_band(prior_mean, prior_std):
    """f32 prior over every offset, exactly as the reference computes it,
    and the band of offsets whose scores can round exp() away from 1.0."""
    d = np.arange(-(N - 1), N, dtype=np.float32)
    ps = np.float32(prior_std)
    pm = np.float32(prior_mean)
    prior = (
        np.float32(1.0)
        / ps
        / np.sqrt(np.float32(2.0) * np.float32(PI))
        * np.exp(np.float32(-0.5) * (d - pm) ** 2 / ps**2)
    ).astype(np.float32)
    # |score| <= |prior| * |q.k*scale| ; bound the latter by 1024 (actual
    # max is ~7 for these glorot inputs).  exp(x) rounds to 1.0f for
    # |x| < 2^-26; use 2^-27 for margin.
    sig = np.abs(prior) * 1024.0 >= 2.0**-27
    if not sig.any():
        dlo, dhi = 0, 0
    else:
        dlo = int(d[sig].min())
        dhi = int(d[sig].max())
    return prior, dlo, dhi


def _window_geometry(dlo, dhi):
    span = dhi - dlo
    win = 128 + span + 16
    win = max(160, ((win + 31) // 32) * 32)
    assert win <= 512, f"prior band too wide for banded kernel: {dlo}..{dhi}"
    extra = win - (128 + span)
    ws_list = []
    for t in range(NT):
        ws = min(max(t * P + dlo - extra // 2, 0), N - win)
        lo_need = max(0, t * P + dlo)
        hi_need = min(N - 1, t * P + 127 + dhi)
        assert ws <= lo_need and hi_need < ws + win, (t, ws, lo_need, hi_need)
        ws_list.append(ws)
    off_vals = sorted({t * P - ws_list[t] for t in range(NT)})
    off_idx = [off_vals.index(t * P - ws_list[t]) for t in range(NT)]
    return win, ws_list, off_vals, off_idx


def _build(win, ws_list, off_idx, n_off):
    nc = bacc.Bacc()

    # consts layout (f32): bq | bk | pmat | j0 | c1 | wsm | ii
    CW = 2 + n_off * win + win + 3 * NT
    O_BQ, O_BK = 0, 1
    O_PM = 2
    O_J0 = O_PM + n_off * win
    O_C1 = O_J0 + win
    O_WS = O_C1 + NT
    O_II = O_WS + NT

    w2_d = nc.dram_tensor("w2", [P, 2 * DCH * MD], BF16, kind="ExternalInput")
    xt_d = nc.dram_tensor("xt", [NPC, P, DCH * PROJ_CHUNK], BF16, kind="ExternalInput")
    cs_d = nc.dram_tensor("cst", [P, CW], F32, kind="ExternalInput")
    y_d = nc.dram_tensor("y", [P, NT], F32, kind="ExternalOutput")

    with tile.TileContext(nc) as tc:
        with (
            tc.tile_pool(name="const", bufs=1) as const,
            tc.tile_pool(name="psum_proj", bufs=3, space="PSUM") as psum_proj,
            tc.tile_pool(name="psum_band", bufs=4, space="PSUM") as psum_band,
            tc.tile_pool(name="band_sp", bufs=3) as sp_pool,
            tc.tile_pool(name="band_e", bufs=3) as e_pool,
            tc.tile_pool(name="band_ec", bufs=3) as ec_pool,
            tc.tile_pool(name="comb", bufs=1) as comb,
        ):
            # ---- input DMAs (weights first: first matmul needs them) ----
            w2_s = const.tile([P, 2 * DCH * MD], BF16, tag="w2")
            nc.sync.dma_start(out=w2_s, in_=w2_d[:, :])
            xts = []
            for i in range(NPC):
                t = const.tile([P, DCH * PROJ_CHUNK], BF16, tag=f"xt{i}")
                nc.sync.dma_start(out=t, in_=xt_d[i])
                xts.append(t)
            cs_s = const.tile([P, CW], F32, tag="cst")
            nc.sync.dma_start(out=cs_s, in_=cs_d[:, :])

            qT = const.tile([P, N], BF16, tag="qT")
            kT = const.tile([P, N], BF16, tag="kT")
            sum_e = const.tile([P, NT], F32, tag="sum_e")
            sum_ec = const.tile([P, NT], F32, tag="sum_ec")

            # ---- projections: qT = Wq.T @ x.T (+bq), kT likewise ----
            for pj in range(2):  # 0=q, 1=k
                b_s = cs_s[:, O_BQ + pj : O_BQ + pj + 1]
                dstT = (qT, kT)[pj]
                for n4 in range(NPC):
                    ps_t = psum_proj.tile([P, PROJ_CHUNK], F32, tag="proj")
                    for c in range(DCH):
                        nc.tensor.matmul(
                            ps_t,
                            lhsT=w2_s[:, (2 * pj + c) * MD : (2 * pj + c + 1) * MD],
                            rhs=xts[n4][:, c * PROJ_CHUNK : (c + 1) * PROJ_CHUNK],
                            start=(c == 0),
                            stop=(c == DCH - 1),
                        )
                    nc.scalar.activation(
                        out=dstT[:, n4 * PROJ_CHUNK : (n4 + 1) * PROJ_CHUNK],
                        in_=ps_t,
                        func=mybir.ActivationFunctionType.Identity,
                        bias=b_s,
                        scale=1.0,
                    )

            # ---- banded scores + softmax partial sums ----
            for t in range(NT):
                ws = ws_list[t]
                oi = off_idx[t]
                ps_s = psum_band.tile([P, win], F32, tag="band")
                nc.tensor.matmul(
                    ps_s,
                    lhsT=qT[:, t * P : (t + 1) * P],
                    rhs=kT[:, ws : ws + win],
                    start=True,
                    stop=True,
                )
                sp_t = sp_pool.tile([P, win], F32, tag="sp")
                nc.vector.tensor_mul(
                    sp_t, ps_s, cs_s[:, O_PM + oi * win : O_PM + (oi + 1) * win]
                )
                e_t = e_pool.tile([P, win], F32, tag="e")
                nc.scalar.activation(
                    out=e_t,
                    in_=sp_t,
                    func=mybir.ActivationFunctionType.Exp,
                    accum_out=sum_e[:, t : t + 1],
                )
                ec_t = ec_pool.tile([P, win], F32, tag="ec")
                nc.vector.affine_mul_reduce(
                    out=ec_t,
                    accum_out=sum_ec[:, t : t + 1],
                    in0=e_t,
                    in1=cs_s[:, O_J0 : O_J0 + win],
                    scale=1.0,
                    bias=0.0,
                )

            # ---- combine: out = (c1 + sum_ec + ws*sum_e)/(N-win+sum_e) - i ----
            c1_s = cs_s[:, O_C1 : O_C1 + NT]
            ws_s = cs_s[:, O_WS : O_WS + NT]
            ii_s = cs_s[:, O_II : O_II + NT]
            t0 = comb.tile([P, NT], F32, tag="t0")
            nc.vector.tensor_scalar_add(t0, sum_e, float(N - win))
            rec = comb.tile([P, NT], F32, tag="rec")
            nc.vector.reciprocal(rec, t0)
            tmp = comb.tile([P, NT], F32, tag="tmp")
            nc.vector.tensor_mul(tmp, ws_s, sum_e)
            num = comb.tile([P, NT], F32, tag="num")
            nc.vector.tensor_add(num, c1_s, sum_ec)
            num2 = comb.tile([P, NT], F32, tag="num2")
            nc.vector.tensor_add(num2, num, tmp)
            outv = comb.tile([P, NT], F32, tag="outv")
            nc.vector.tensor_mul(outv, num2, rec)
            outv2 = comb.tile([P, NT], F32, tag="outv2")
            nc.vector.tensor_sub(outv2, outv, ii_s)

            nc.sync.dma_start(out=y_d[:, :], in_=outv2)

    nc.finalize()
    return nc


def kernel(x, Wq, bq, Wk, bk, prior_mean, prior_std):
    global last_run
    x = np.asarray(x, dtype=np.float32)
    Wq = np.asarray(Wq, dtype=np.float32)
    Wk = np.asarray(Wk, dtype=np.float32)
    bq = np.asarray(bq, dtype=np.float32)
    bk = np.asarray(bk, dtype=np.float32)

    prior, dlo, dhi = _plan_band(
        float(np.asarray(prior_mean)[0]), float(np.asarray(prior_std)[0])
    )
    win, ws_list, off_vals, off_idx = _window_geometry(dlo, dhi)
    n_off = len(off_vals)

    key = (win, tuple(ws_list), tuple(off_idx))
    if key not in _cache:
        _cache[key] = _build(win, ws_list, off_idx, n_off)
    nc = _cache[key]

    bf = ml_dtypes.bfloat16
    scale = np.float32(MD**-0.5)

    # prior*scale window matrices, one per distinct tile offset
    r = np.arange(P)[:, None]
    c = np.arange(win)[None, :]
    pmat = np.zeros((P, n_off * win), np.float32)
    for k_i, off in enumerate(off_vals):
        dm = c - off - r
        pmat[:, k_i * win : (k_i + 1) * win] = np.where(
            (dm >= dlo) & (dm <= dhi), prior[dm + N - 1] * scale, np.float32(0.0)
        ).astype(np.float32)

    sumj_all = float(N * (N - 1) // 2)
    c1 = np.zeros((P, NT), np.float32)
    wsm = np.zeros((P, NT), np.float32)
    ii = np.zeros((P, NT), np.float32)
    for t in range(NT):
        ws = ws_list[t]
        c1[:, t] = sumj_all - (win * ws + win * (win - 1) // 2)
        wsm[:, t] = ws
        ii[:, t] = t * P + np.arange(P)

    # consts tensor: bq | bk | pmat | j0 | c1 | wsm | ii
    j0 = np.broadcast_to(np.arange(win, dtype=np.float32), (P, win))
    cst = np.concatenate(
        [bq.reshape(P, 1), bk.reshape(P, 1), pmat, j0, c1, wsm, ii], axis=1
    ).astype(np.float32)
    cst = np.ascontiguousarray(cst)

    # weights: wq chunks then wk chunks, [P, 4*MD]
    wq_h = Wq.reshape(DCH, P, MD).transpose(1, 0, 2).reshape(P, DCH * MD)
    wk_h = Wk.reshape(DCH, P, MD).transpose(1, 0, 2).reshape(P, DCH * MD)
    w2_h = np.ascontiguousarray(np.concatenate([wq_h, wk_h], axis=1)).astype(bf)

    in_maps = []
    for core in range(NCORES):
        xb = x[core]  # [N, D]
        # xt[n4, p, c*512 + j] = x[n4*512 + j, c*128 + p]
        xt_h = np.ascontiguousarray(
            xb.T.reshape(DCH, P, NPC, PROJ_CHUNK)  # [c, p, n4, j]
            .transpose(2, 1, 0, 3)  # [n4, p, c, j]
            .reshape(NPC, P, DCH * PROJ_CHUNK)
        ).astype(bf)
        in_maps.append({"xt": xt_h, "w2": w2_h, "cst": cst})

    res = run_bass_kernel_spmd(nc, in_maps, list(range(NCORES)))
    last_run = (nc, in_maps)
    # y[p, t] = out[128t + p]  ->  out = y.T.flatten()
    out = np.stack(
        [res.results[c]["y"].T.reshape(-1) for c in range(NCORES)], axis=0
    )
    return out.astype(np.float32)


# revision 6
# speedup vs baseline: 1.4326x; 1.1005x over previous
"""Trainium2 Bass kernel for nn_DistanceLayer (gaussian-prior distance attention).

Math: out[b,i] = sum_j softmax_j(q_i.k_j * MD^-0.5 * prior(j-i))[j] * (j-i)

Key observation: the gaussian prior (std=1) underflows so fast in f32 that
for |j-i| outside a small band the f32 score is exactly 0, so exp(score)
is exactly 1.0.  The softmax row then consists of a small band of
"interesting" values plus a uniform far field whose sums are known in
closed form.  We therefore compute, per 128-row tile, only a small window
of scores around the diagonal on the PE, and fold the far field in with
exact host-side constants:

    T0_i = (N - win) + sum_window exp(s)            (denominator)
    T1_i = C1_t + sum_window exp(s)*c + ws_t * sum_window exp(s)
    out_i = T1_i / T0_i - i

where C1_t = sum_all_j j - sum_window j (exact integers < 2^24, exact in
f32) and ws_t is the window start of tile t.  In-window far entries have
score exactly 0 (prior premultiplied in, 0 outside band) and contribute
exp(0)=1 which the constants account for.

Sharding: pure data-parallel over batch B=8 across the 8 cores; each core
holds the full (small) QK weights and computes its own [N] output row.
"""

import sys

sys.path.insert(0, "/opt/trn_rl_repo")

import ml_dtypes
import numpy as np

import concourse.bacc as bacc
import concourse.tile as tile
from concourse import mybir
from concourse.bass_utils import run_bass_kernel_spmd

B, N, D, MD = 8, 2048, 256, 128
NCORES = 8
P = 128
NT = N // P  # 16 row tiles
DCH = D // P  # 2 contraction chunks for the projections
PROJ_CHUNK = 512
NPC = N // PROJ_CHUNK  # 4 projection column chunks
PI = 3.1415926  # matches reference
F32 = mybir.dt.float32
BF16 = mybir.dt.bfloat16

_cache = {}
# exposed for test harness profiling: (nc, in_maps)
last_run = None


def _plan_band(prior_mean, prior_std):
    """f32 prior over every offset, exactly as the reference computes it,
    and the band of offsets whose scores can round exp() away from 1.0."""
    d = np.arange(-(N - 1), N, dtype=np.float32)
    ps = np.float32(prior_std)
    pm = np.float32(prior_mean)
    prior = (
        np.float32(1.0)
        / ps
        / np.sqrt(np.float32(2.0) * np.float32(PI))
        * np.exp(np.float32(-0.5) * (d - pm) ** 2 / ps**2)
    ).astype(np.float32)
    # |score| <= |prior| * |q.k*scale| ; bound the latter by 1024 (actual
    # max is ~7 for these glorot inputs).  exp(x) rounds to 1.0f for
    # |x| < 2^-26; use 2^-27 for margin.
    sig = np.abs(prior) * 1024.0 >= 2.0**-27
    if not sig.any():
        dlo, dhi = 0, 0
    else:
        dlo = int(d[sig].min())
        dhi = int(d[sig].max())
    return prior, dlo, dhi


def _window_geometry(dlo, dhi):
    span = dhi - dlo
    win = 128 + span + 1
    win = max(144, ((win + 15) // 16) * 16)
    assert win <= 512, f"prior band too wide for banded kernel: {dlo}..{dhi}"
    extra = win - (128 + span)
    ws_list = []
    for t in range(NT):
        ws = min(max(t * P + dlo - extra // 2, 0), N - win)
        lo_need = max(0, t * P + dlo)
        hi_need = min(N - 1, t * P + 127 + dhi)
        assert ws <= lo_need and hi_need < ws + win, (t, ws, lo_need, hi_need)
        ws_list.append(ws)
    off_vals = sorted({t * P - ws_list[t] for t in range(NT)})
    off_idx = [off_vals.index(t * P - ws_list[t]) for t in range(NT)]
    return win, ws_list, off_vals, off_idx


def _build(win, ws_list, off_idx, n_off):
    nc = bacc.Bacc()

    # consts layout (f32): bq | bk | pmat | j0 | c1 | wsm | ii
    CW = 2 + n_off * win + win + 3 * NT
    O_BQ, O_BK = 0, 1
    O_PM = 2
    O_J0 = O_PM + n_off * win
    O_C1 = O_J0 + win
    O_WS = O_C1 + NT
    O_II = O_WS + NT

    w2_d = nc.dram_tensor("w2", [P, 2 * DCH * MD], BF16, kind="ExternalInput")
    xt_d = nc.dram_tensor("xt", [NPC, P, DCH * PROJ_CHUNK], BF16, kind="ExternalInput")
    cs_d = nc.dram_tensor("cst", [P, CW], F32, kind="ExternalInput")
    y_d = nc.dram_tensor("y", [P, NT], F32, kind="ExternalOutput")

    with tile.TileContext(nc) as tc:
        with (
            tc.tile_pool(name="const", bufs=1) as const,
            tc.tile_pool(name="psum_proj", bufs=3, space="PSUM") as psum_proj,
            tc.tile_pool(name="psum_band", bufs=4, space="PSUM") as psum_band,
            tc.tile_pool(name="band_sp", bufs=3) as sp_pool,
            tc.tile_pool(name="band_e", bufs=3) as e_pool,
            tc.tile_pool(name="band_ec", bufs=3) as ec_pool,
            tc.tile_pool(name="comb", bufs=1) as comb,
        ):
            # ---- engine warmups (run while DMAs are in flight) ----
            # PE: ~8 junk matmuls (~4us busy) flip the HAM clock gate to
            # 8/8 before the real matmuls arrive.  ACT: one tiny Exp pulls
            # the 1.3us ACT_TABLE_LOAD off the critical path.
            wtile = const.tile([P, win], BF16, tag="warm_w")
            nc.vector.memset(wtile, 0.0)
            for _ in range(8):
                wps = psum_band.tile([P, win], F32, tag="band")
                nc.tensor.matmul(
                    wps, lhsT=wtile[:, :P], rhs=wtile[:, :win], start=True, stop=True
                )
            wact_in = const.tile([P, 1], F32, tag="warm_a")
            nc.vector.memset(wact_in, 0.0)
            wact_out = const.tile([P, 1], F32, tag="warm_ao")
            nc.scalar.activation(
                out=wact_out, in_=wact_in, func=mybir.ActivationFunctionType.Exp
            )

            # ---- input DMAs (weights first: first matmul needs them) ----
            w2_s = const.tile([P, 2 * DCH * MD], BF16, tag="w2")
            nc.sync.dma_start(out=w2_s, in_=w2_d[:, :])
            xts = []
            for i in range(NPC):
                t = const.tile([P, DCH * PROJ_CHUNK], BF16, tag=f"xt{i}")
                xts.append(t)
            nc.sync.dma_start(out=xts[0], in_=xt_d[0])
            cs_s = const.tile([P, CW], F32, tag="cst")
            nc.sync.dma_start(out=cs_s, in_=cs_d[:, :])
            for i in range(1, NPC):
                nc.sync.dma_start(out=xts[i], in_=xt_d[i])

            qT = const.tile([P, N], BF16, tag="qT")
            kT = const.tile([P, N], BF16, tag="kT")
            sum_e = const.tile([P, NT], F32, tag="sum_e")
            sum_ec = const.tile([P, NT], F32, tag="sum_ec")

            # ---- projections: qT = Wq.T @ x.T (+bq), kT likewise ----
            # n4-major with q/k interleaved so band tile t can start as soon
            # as chunk t//4 of both projections is evicted.  q evictions on
            # DVE, k evictions on ACT (both near-idle during this phase).
            for n4 in range(NPC):
                for pj in range(2):  # 0=q, 1=k
                    b_s = cs_s[:, O_BQ + pj : O_BQ + pj + 1]
                    dstT = (qT, kT)[pj]
                    ps_t = psum_proj.tile([P, PROJ_CHUNK], F32, tag="proj")
                    for c in range(DCH):
                        nc.tensor.matmul(
                            ps_t,
                            lhsT=w2_s[:, (2 * pj + c) * MD : (2 * pj + c + 1) * MD],
                            rhs=xts[n4][:, c * PROJ_CHUNK : (c + 1) * PROJ_CHUNK],
                            start=(c == 0),
                            stop=(c == DCH - 1),
                        )
                    dst_slice = dstT[:, n4 * PROJ_CHUNK : (n4 + 1) * PROJ_CHUNK]
                    if pj == 0:
                        nc.vector.tensor_scalar_add(dst_slice, ps_t, b_s)
                    else:
                        nc.scalar.activation(
                            out=dst_slice,
                            in_=ps_t,
                            func=mybir.ActivationFunctionType.Identity,
                            bias=b_s,
                            scale=1.0,
                        )

            # ---- banded scores + softmax partial sums ----
            for t in range(NT):
                ws = ws_list[t]
                oi = off_idx[t]
                ps_s = psum_band.tile([P, win], F32, tag="band")
                nc.tensor.matmul(
                    ps_s,
                    lhsT=qT[:, t * P : (t + 1) * P],
                    rhs=kT[:, ws : ws + win],
                    start=True,
                    stop=True,
                )
                sp_t = sp_pool.tile([P, win], F32, tag="sp")
                nc.vector.tensor_mul(
                    sp_t, ps_s, cs_s[:, O_PM + oi * win : O_PM + (oi + 1) * win]
                )
                e_t = e_pool.tile([P, win], F32, tag="e")
                nc.scalar.activation(
                    out=e_t,
                    in_=sp_t,
                    func=mybir.ActivationFunctionType.Exp,
                    accum_out=sum_e[:, t : t + 1],
                )
                ec_t = ec_pool.tile([P, win], F32, tag="ec")
                nc.vector.affine_mul_reduce(
                    out=ec_t,
                    accum_out=sum_ec[:, t : t + 1],
                    in0=e_t,
                    in1=cs_s[:, O_J0 : O_J0 + win],
                    scale=1.0,
                    bias=0.0,
                )

            # ---- combine: out = (c1 + sum_ec + ws*sum_e)/(N-win+sum_e) - i ----
            c1_s = cs_s[:, O_C1 : O_C1 + NT]
            ws_s = cs_s[:, O_WS : O_WS + NT]
            ii_s = cs_s[:, O_II : O_II + NT]
            t0 = comb.tile([P, NT], F32, tag="t0")
            nc.vector.tensor_scalar_add(t0, sum_e, float(N - win))
            rec = comb.tile([P, NT], F32, tag="rec")
            nc.vector.reciprocal(rec, t0)
            tmp = comb.tile([P, NT], F32, tag="tmp")
            nc.vector.tensor_mul(tmp, ws_s, sum_e)
            num = comb.tile([P, NT], F32, tag="num")
            nc.vector.tensor_add(num, c1_s, sum_ec)
            num2 = comb.tile([P, NT], F32, tag="num2")
            nc.vector.tensor_add(num2, num, tmp)
            outv = comb.tile([P, NT], F32, tag="outv")
            nc.vector.tensor_mul(outv, num2, rec)
            outv2 = comb.tile([P, NT], F32, tag="outv2")
            nc.vector.tensor_sub(outv2, outv, ii_s)

            nc.sync.dma_start(out=y_d[:, :], in_=outv2)

    nc.finalize()
    return nc


def kernel(x, Wq, bq, Wk, bk, prior_mean, prior_std):
    global last_run
    x = np.asarray(x, dtype=np.float32)
    Wq = np.asarray(Wq, dtype=np.float32)
    Wk = np.asarray(Wk, dtype=np.float32)
    bq = np.asarray(bq, dtype=np.float32)
    bk = np.asarray(bk, dtype=np.float32)

    prior, dlo, dhi = _plan_band(
        float(np.asarray(prior_mean)[0]), float(np.asarray(prior_std)[0])
    )
    win, ws_list, off_vals, off_idx = _window_geometry(dlo, dhi)
    n_off = len(off_vals)

    key = (win, tuple(ws_list), tuple(off_idx))
    if key not in _cache:
        _cache[key] = _build(win, ws_list, off_idx, n_off)
    nc = _cache[key]

    bf = ml_dtypes.bfloat16
    scale = np.float32(MD**-0.5)

    # prior*scale window matrices, one per distinct tile offset
    r = np.arange(P)[:, None]
    c = np.arange(win)[None, :]
    pmat = np.zeros((P, n_off * win), np.float32)
    for k_i, off in enumerate(off_vals):
        dm = c - off - r
        pmat[:, k_i * win : (k_i + 1) * win] = np.where(
            (dm >= dlo) & (dm <= dhi), prior[dm + N - 1] * scale, np.float32(0.0)
        ).astype(np.float32)

    sumj_all = float(N * (N - 1) // 2)
    c1 = np.zeros((P, NT), np.float32)
    wsm = np.zeros((P, NT), np.float32)
    ii = np.zeros((P, NT), np.float32)
    for t in range(NT):
        ws = ws_list[t]
        c1[:, t] = sumj_all - (win * ws + win * (win - 1) // 2)
        wsm[:, t] = ws
        ii[:, t] = t * P + np.arange(P)

    # consts tensor: bq | bk | pmat | j0 | c1 | wsm | ii
    j0 = np.broadcast_to(np.arange(win, dtype=np.float32), (P, win))
    cst = np.concatenate(
        [bq.reshape(P, 1), bk.reshape(P, 1), pmat, j0, c1, wsm, ii], axis=1
    ).astype(np.float32)
    cst = np.ascontiguousarray(cst)

    # weights: wq chunks then wk chunks, [P, 4*MD]
    wq_h = Wq.reshape(DCH, P, MD).transpose(1, 0, 2).reshape(P, DCH * MD)
    wk_h = Wk.reshape(DCH, P, MD).transpose(1, 0, 2).reshape(P, DCH * MD)
    w2_h = np.ascontiguousarray(np.concatenate([wq_h, wk_h], axis=1)).astype(bf)

    in_maps = []
    for core in range(NCORES):
        xb = x[core]  # [N, D]
        # xt[n4, p, c*512 + j] = x[n4*512 + j, c*128 + p]
        xt_h = np.ascontiguousarray(
            xb.T.reshape(DCH, P, NPC, PROJ_CHUNK)  # [c, p, n4, j]
            .transpose(2, 1, 0, 3)  # [n4, p, c, j]
            .reshape(NPC, P, DCH * PROJ_CHUNK)
        ).astype(bf)
        in_maps.append({"xt": xt_h, "w2": w2_h, "cst": cst})

    res = run_bass_kernel_spmd(nc, in_maps, list(range(NCORES)))
    last_run = (nc, in_maps)
    # y[p, t] = out[128t + p]  ->  out = y.T.flatten()
    out = np.stack(
        [res.results[c]["y"].T.reshape(-1) for c in range(NCORES)], axis=0
    )
    return out.astype(np.float32)


# revision 8
# speedup vs baseline: 1.5222x; 1.0626x over previous
"""Trainium2 Bass kernel for nn_DistanceLayer (gaussian-prior distance attention).

Math: out[b,i] = sum_j softmax_j(q_i.k_j * MD^-0.5 * prior(j-i))[j] * (j-i)

Key observation: the gaussian prior (std=1) underflows so fast in f32 that
for |j-i| outside a small band the f32 score is exactly 0, so exp(score)
is exactly 1.0.  The softmax row then consists of a small band of
"interesting" values plus a uniform far field whose sums are known in
closed form.  We therefore compute, per 128-row tile, only a small window
of scores around the diagonal on the PE, and fold the far field in with
exact host-side constants:

    T0_i = (N - win) + sum_window exp(s)            (denominator)
    T1_i = C1_t + sum_window exp(s)*c + ws_t * sum_window exp(s)
    out_i = T1_i / T0_i - i

where C1_t = sum_all_j j - sum_window j (exact integers < 2^24, exact in
f32) and ws_t is the window start of tile t.  In-window far entries have
score exactly 0 (prior premultiplied in, 0 outside band) and contribute
exp(0)=1 which the constants account for.

Sharding: pure data-parallel over batch B=8 across the 8 cores; each core
holds the full (small) QK weights and computes its own [N] output row.
"""

import sys

sys.path.insert(0, "/opt/trn_rl_repo")

import ml_dtypes
import numpy as np

import concourse.bacc as bacc
import concourse.tile as tile
from concourse import mybir
from concourse.bass_utils import run_bass_kernel_spmd

B, N, D, MD = 8, 2048, 256, 128
NCORES = 8
P = 128
NT = N // P  # 16 row tiles
DCH = D // P  # 2 contraction chunks for the projections
PROJ_CHUNK = 512
NPC = N // PROJ_CHUNK  # 4 projection column chunks
PI = 3.1415926  # matches reference
F32 = mybir.dt.float32
BF16 = mybir.dt.bfloat16

_cache = {}
# exposed for test harness profiling: (nc, in_maps)
last_run = None


def _plan_band(prior_mean, prior_std):
    """f32 prior over every offset, exactly as the reference computes it,
    and the band of offsets whose scores can round exp() away from 1.0."""
    d = np.arange(-(N - 1), N, dtype=np.float32)
    ps = np.float32(prior_std)
    pm = np.float32(prior_mean)
    prior = (
        np.float32(1.0)
        / ps
        / np.sqrt(np.float32(2.0) * np.float32(PI))
        * np.exp(np.float32(-0.5) * (d - pm) ** 2 / ps**2)
    ).astype(np.float32)
    # |score| <= |prior| * |q.k*scale| ; bound the latter by 1024 (actual
    # max is ~7 for these glorot inputs).  exp(x) rounds to 1.0f for
    # |x| < 2^-26; use 2^-27 for margin.
    sig = np.abs(prior) * 1024.0 >= 2.0**-27
    if not sig.any():
        dlo, dhi = 0, 0
    else:
        dlo = int(d[sig].min())
        dhi = int(d[sig].max())
    return prior, dlo, dhi


def _window_geometry(dlo, dhi):
    span = dhi - dlo
    win = 128 + span + 1
    win = max(144, ((win + 15) // 16) * 16)
    assert win <= 512, f"prior band too wide for banded kernel: {dlo}..{dhi}"
    extra = win - (128 + span)
    ws_list = []
    for t in range(NT):
        ws = min(max(t * P + dlo - extra // 2, 0), N - win)
        lo_need = max(0, t * P + dlo)
        hi_need = min(N - 1, t * P + 127 + dhi)
        assert ws <= lo_need and hi_need < ws + win, (t, ws, lo_need, hi_need)
        ws_list.append(ws)
    off_vals = sorted({t * P - ws_list[t] for t in range(NT)})
    off_idx = [off_vals.index(t * P - ws_list[t]) for t in range(NT)]
    return win, ws_list, off_vals, off_idx


def _build(win, ws_list, off_idx, n_off):
    nc = bacc.Bacc()

    # consts layout (f32): bq | bk | pmat | j0 | c1 | wsm | ii
    CW = 2 + n_off * win + win + 3 * NT
    O_BQ, O_BK = 0, 1
    O_PM = 2
    O_J0 = O_PM + n_off * win
    O_C1 = O_J0 + win
    O_WS = O_C1 + NT
    O_II = O_WS + NT

    w2_d = nc.dram_tensor("w2", [P, 2 * DCH * MD], BF16, kind="ExternalInput")
    xt_d = nc.dram_tensor("xt", [NPC, P, DCH * PROJ_CHUNK], BF16, kind="ExternalInput")
    cs_d = nc.dram_tensor("cst", [P, CW], F32, kind="ExternalInput")
    y_d = nc.dram_tensor("y", [P, NT], F32, kind="ExternalOutput")

    with tile.TileContext(nc) as tc:
        with (
            tc.tile_pool(name="const", bufs=1) as const,
            tc.tile_pool(name="psum_proj", bufs=3, space="PSUM") as psum_proj,
            tc.tile_pool(name="psum_band", bufs=4, space="PSUM") as psum_band,
            tc.tile_pool(name="band_sp", bufs=3) as sp_pool,
            tc.tile_pool(name="band_e", bufs=3) as e_pool,
            tc.tile_pool(name="band_ec", bufs=3) as ec_pool,
            tc.tile_pool(name="comb", bufs=1) as comb,
        ):
            # ---- engine warmups (run while DMAs are in flight) ----
            # PE: ~8 junk matmuls (~4us busy) flip the HAM clock gate to
            # 8/8 before the real matmuls arrive.  ACT: one tiny Exp pulls
            # the 1.3us ACT_TABLE_LOAD off the critical path.
            wtile = const.tile([P, win], BF16, tag="warm_w")
            nc.vector.memset(wtile, 0.0)
            for _ in range(28):
                wps = psum_band.tile([P, win], F32, tag="band")
                nc.tensor.matmul(
                    wps, lhsT=wtile[:, :P], rhs=wtile[:, :win], start=True, stop=True
                )
            wact_in = const.tile([P, 1], F32, tag="warm_a")
            nc.vector.memset(wact_in, 0.0)
            wact_out = const.tile([P, 1], F32, tag="warm_ao")
            nc.scalar.activation(
                out=wact_out, in_=wact_in, func=mybir.ActivationFunctionType.Exp
            )

            # ---- input DMAs (weights first: first matmul needs them) ----
            w2_s = const.tile([P, 2 * DCH * MD], BF16, tag="w2")
            nc.sync.dma_start(out=w2_s, in_=w2_d[:, :])
            xts = []
            for i in range(NPC):
                t = const.tile([P, DCH * PROJ_CHUNK], BF16, tag=f"xt{i}")
                xts.append(t)
            nc.sync.dma_start(out=xts[0], in_=xt_d[0])
            cs_s = const.tile([P, CW], F32, tag="cst")
            nc.sync.dma_start(out=cs_s, in_=cs_d[:, :])
            for i in range(1, NPC):
                nc.sync.dma_start(out=xts[i], in_=xt_d[i])

            qT = const.tile([P, N], BF16, tag="qT")
            kT = const.tile([P, N], BF16, tag="kT")
            sum_e = const.tile([P, NT], F32, tag="sum_e")
            sum_ec = const.tile([P, NT], F32, tag="sum_ec")

            # ---- projections: qT = Wq.T @ x.T (+bq), kT likewise ----
            # n4-major with q/k interleaved so band tile t can start as soon
            # as chunk t//4 of both projections is evicted.  q evictions on
            # DVE, k evictions on ACT (both near-idle during this phase).
            for n4 in range(NPC):
                for pj in range(2):  # 0=q, 1=k
                    b_s = cs_s[:, O_BQ + pj : O_BQ + pj + 1]
                    dstT = (qT, kT)[pj]
                    ps_t = psum_proj.tile([P, PROJ_CHUNK], F32, tag="proj")
                    for c in range(DCH):
                        nc.tensor.matmul(
                            ps_t,
                            lhsT=w2_s[:, (2 * pj + c) * MD : (2 * pj + c + 1) * MD],
                            rhs=xts[n4][:, c * PROJ_CHUNK : (c + 1) * PROJ_CHUNK],
                            start=(c == 0),
                            stop=(c == DCH - 1),
                        )
                    # evict halves on DVE and ACT concurrently
                    half = PROJ_CHUNK // 2
                    lo = n4 * PROJ_CHUNK
                    nc.vector.tensor_scalar_add(
                        dstT[:, lo : lo + half], ps_t[:, :half], b_s
                    )
                    nc.scalar.activation(
                        out=dstT[:, lo + half : lo + PROJ_CHUNK],
                        in_=ps_t[:, half:],
                        func=mybir.ActivationFunctionType.Identity,
                        bias=b_s,
                        scale=1.0,
                    )

            # ---- banded scores + softmax partial sums ----
            for t in range(NT):
                ws = ws_list[t]
                oi = off_idx[t]
                ps_s = psum_band.tile([P, win], F32, tag="band")
                nc.tensor.matmul(
                    ps_s,
                    lhsT=qT[:, t * P : (t + 1) * P],
                    rhs=kT[:, ws : ws + win],
                    start=True,
                    stop=True,
                )
                sp_t = sp_pool.tile([P, win], F32, tag="sp")
                nc.vector.tensor_mul(
                    sp_t, ps_s, cs_s[:, O_PM + oi * win : O_PM + (oi + 1) * win]
                )
                e_t = e_pool.tile([P, win], F32, tag="e")
                nc.scalar.activation(
                    out=e_t,
                    in_=sp_t,
                    func=mybir.ActivationFunctionType.Exp,
                    accum_out=sum_e[:, t : t + 1],
                )
                ec_t = ec_pool.tile([P, win], F32, tag="ec")
                nc.vector.affine_mul_reduce(
                    out=ec_t,
                    accum_out=sum_ec[:, t : t + 1],
                    in0=e_t,
                    in1=cs_s[:, O_J0 : O_J0 + win],
                    scale=1.0,
                    bias=0.0,
                )

            # ---- combine: out = (c1 + sum_ec + ws*sum_e)/(N-win+sum_e) - i ----
            c1_s = cs_s[:, O_C1 : O_C1 + NT]
            ws_s = cs_s[:, O_WS : O_WS + NT]
            ii_s = cs_s[:, O_II : O_II + NT]
            t0 = comb.tile([P, NT], F32, tag="t0")
            nc.vector.tensor_scalar_add(t0, sum_e, float(N - win))
            rec = comb.tile([P, NT], F32, tag="rec")
            nc.vector.reciprocal(rec, t0)
            tmp = comb.tile([P, NT], F32, tag="tmp")
            nc.vector.tensor_mul(tmp, ws_s, sum_e)
            num = comb.tile([P, NT], F32, tag="num")
            nc.vector.tensor_add(num, c1_s, sum_ec)
            num2 = comb.tile([P, NT], F32, tag="num2")
            nc.vector.tensor_add(num2, num, tmp)
            outv = comb.tile([P, NT], F32, tag="outv")
            nc.vector.tensor_mul(outv, num2, rec)
            outv2 = comb.tile([P, NT], F32, tag="outv2")
            nc.vector.tensor_sub(outv2, outv, ii_s)

            nc.sync.dma_start(out=y_d[:, :], in_=outv2)

    nc.finalize()
    return nc


def kernel(x, Wq, bq, Wk, bk, prior_mean, prior_std):
    global last_run
    x = np.asarray(x, dtype=np.float32)
    Wq = np.asarray(Wq, dtype=np.float32)
    Wk = np.asarray(Wk, dtype=np.float32)
    bq = np.asarray(bq, dtype=np.float32)
    bk = np.asarray(bk, dtype=np.float32)

    prior, dlo, dhi = _plan_band(
        float(np.asarray(prior_mean)[0]), float(np.asarray(prior_std)[0])
    )
    win, ws_list, off_vals, off_idx = _window_geometry(dlo, dhi)
    n_off = len(off_vals)

    key = (win, tuple(ws_list), tuple(off_idx))
    if key not in _cache:
        _cache[key] = _build(win, ws_list, off_idx, n_off)
    nc = _cache[key]

    bf = ml_dtypes.bfloat16
    scale = np.float32(MD**-0.5)

    # prior*scale window matrices, one per distinct tile offset
    r = np.arange(P)[:, None]
    c = np.arange(win)[None, :]
    pmat = np.zeros((P, n_off * win), np.float32)
    for k_i, off in enumerate(off_vals):
        dm = c - off - r
        pmat[:, k_i * win : (k_i + 1) * win] = np.where(
            (dm >= dlo) & (dm <= dhi), prior[dm + N - 1] * scale, np.float32(0.0)
        ).astype(np.float32)

    sumj_all = float(N * (N - 1) // 2)
    c1 = np.zeros((P, NT), np.float32)
    wsm = np.zeros((P, NT), np.float32)
    ii = np.zeros((P, NT), np.float32)
    for t in range(NT):
        ws = ws_list[t]
        c1[:, t] = sumj_all - (win * ws + win * (win - 1) // 2)
        wsm[:, t] = ws
        ii[:, t] = t * P + np.arange(P)

    # consts tensor: bq | bk | pmat | j0 | c1 | wsm | ii
    j0 = np.broadcast_to(np.arange(win, dtype=np.float32), (P, win))
    cst = np.concatenate(
        [bq.reshape(P, 1), bk.reshape(P, 1), pmat, j0, c1, wsm, ii], axis=1
    ).astype(np.float32)
    cst = np.ascontiguousarray(cst)

    # weights: wq chunks then wk chunks, [P, 4*MD]
    wq_h = Wq.reshape(DCH, P, MD).transpose(1, 0, 2).reshape(P, DCH * MD)
    wk_h = Wk.reshape(DCH, P, MD).transpose(1, 0, 2).reshape(P, DCH * MD)
    w2_h = np.ascontiguousarray(np.concatenate([wq_h, wk_h], axis=1)).astype(bf)

    in_maps = []
    for core in range(NCORES):
        xb = x[core]  # [N, D]
        # xt[n4, p, c*512 + j] = x[n4*512 + j, c*128 + p]
        xt_h = np.ascontiguousarray(
            xb.T.reshape(DCH, P, NPC, PROJ_CHUNK)  # [c, p, n4, j]
            .transpose(2, 1, 0, 3)  # [n4, p, c, j]
            .reshape(NPC, P, DCH * PROJ_CHUNK)
        ).astype(bf)
        in_maps.append({"xt": xt_h, "w2": w2_h, "cst": cst})

    res = run_bass_kernel_spmd(nc, in_maps, list(range(NCORES)))
    last_run = (nc, in_maps)
    # y[p, t] = out[128t + p]  ->  out = y.T.flatten()
    out = np.stack(
        [res.results[c]["y"].T.reshape(-1) for c in range(NCORES)], axis=0
    )
    return out.astype(np.float32)
